# revision 7
# baseline (speedup 1.0000x reference)
"""Trainium2 Bass kernel: GNN message passing (2x encoder/decoder MLP + NeighDiff).

Sharding: nodes row-sharded across 8 NeuronCores (1024 nodes/core).
The segment-mean NeighDiff is computed via the algebraic expansion
    dna_i = mask_i * (||h_i||^2 + (q_i - 2 h_i . m_i) / c_i)
with [m | q] = B @ [h | hsq], where B is the dense [dst, src] edge-count
matrix (built host-side from edge_index as part of sharding prep) and the
full node embedding table is AllGathered on-device in bf16.
"""

import numpy as np
import ml_dtypes

import concourse.bass as bass
import concourse.mybir as mybir
import concourse.tile as tile
from concourse import bacc
from concourse.bass_utils import run_bass_kernel_spmd
from concourse.masks import make_identity

BF16 = mybir.dt.bfloat16
F32 = mybir.dt.float32

NCORES = 8
N = 8192          # nodes
NL = N // NCORES  # nodes per core (1024)
XD = 256          # x feature dim
H = 128           # hidden dim
SD = 8192         # s feature dim
TD = 2 * H + 2    # allgather table cols: h_a | h_s | hsq_a | hsq_s
KC = N // 128     # k-chunks over source nodes (64)
SC = SD // 128    # k-chunks over s columns (64)

_CACHE = {}


def _build_program():
    nc = bacc.Bacc("TRN2", target_bir_lowering=False, debug=False,
                   num_devices=NCORES)

    def din(name, shape, dt):
        return nc.dram_tensor(name, shape, dt, kind="ExternalInput").ap()

    def dout(name, shape, dt):
        return nc.dram_tensor(name, shape, dt, kind="ExternalOutput").ap()

    # per-core sharded inputs
    xT = din("xT", [XD, NL], BF16)            # x rows, transposed
    sT = din("sT", [SD, NL], BF16)            # s rows, transposed
    BT = din("BT", [N, NL], BF16)             # edge-count matrix [src, dst_local]
    cinv = din("cinv", [1, NL], F32)          # 1/max(deg,1) for local dst nodes
    mask = din("mask", [1, NL], F32)          # deg>0 for local dst nodes
    # replicated weights (bf16) and biases
    ae_W1 = din("ae_W1", [XD, H], BF16)
    ae_W2 = din("ae_W2", [H, H], BF16)
    ad_W1 = din("ad_W1", [H, H], BF16)
    ad_W2 = din("ad_W2", [H, XD], BF16)
    se_W1 = din("se_W1", [SD, H], BF16)
    se_W2 = din("se_W2", [H, H], BF16)
    sd_W1 = din("sd_W1", [H, H], BF16)
    sd_W2 = din("sd_W2", [H, SD], BF16)
    ae_b1 = din("ae_b1", [H, 1], F32)
    ae_b2 = din("ae_b2", [H, 1], F32)
    ad_b1 = din("ad_b1", [H, 1], F32)
    se_b1 = din("se_b1", [H, 1], F32)
    se_b2 = din("se_b2", [H, 1], F32)
    sd_b1 = din("sd_b1", [H, 1], F32)
    ad_b2r = din("ad_b2r", [1, XD], BF16)     # decoder output biases as rows
    sd_b2r = din("sd_b2r", [1, SD], BF16)

    # per-core outputs
    o_x = dout("o_x", [NL, XD], F32)
    o_s = dout("o_s", [NL, SD], BF16)
    o_ha = dout("o_ha", [NL, H], F32)
    o_hs = dout("o_hs", [NL, H], F32)
    o_dna = dout("o_dna", [1, NL], F32)
    o_dns = dout("o_dns", [1, NL], F32)

    with tile.TileContext(nc) as tc:
        _emit(nc, tc, locals())
    nc.compile()
    return nc


def _emit(nc, tc, t):
    Relu = mybir.ActivationFunctionType.Relu
    Ident = mybir.ActivationFunctionType.Identity
    Square = mybir.ActivationFunctionType.Square
    add = mybir.AluOpType.add
    mult = mybir.AluOpType.mult

    import contextlib
    ctx = contextlib.ExitStack()
    ctx1 = contextlib.ExitStack()  # phase-1 PSUM pools, closed before phase 2
    with ctx:
        wp = ctx.enter_context(tc.tile_pool(name="wp", bufs=1))
        io = ctx.enter_context(tc.tile_pool(name="io", bufs=3))
        dram = ctx.enter_context(tc.tile_pool(name="dram", bufs=1, space="DRAM"))
        psA = ctx1.enter_context(tc.tile_pool(name="psA", bufs=2, space="PSUM"))
        psTr = ctx1.enter_context(tc.tile_pool(name="psTr", bufs=2, space="PSUM"))

        # --- critical-path weights first (s-encoder + x-chain) ---
        w_se1 = wp.tile([128, SC * H], BF16)          # se_W1 k-chunks side by side
        for k in range(SC):
            nc.sync.dma_start(out=w_se1[:, k * H:(k + 1) * H],
                              in_=t["se_W1"][k * 128:(k + 1) * 128, :])
        w_ae1 = wp.tile([128, 2 * H], BF16)
        for k in range(2):
            nc.sync.dma_start(out=w_ae1[:, k * H:(k + 1) * H],
                              in_=t["ae_W1"][k * 128:(k + 1) * 128, :])
        w_small = {}
        for nm in ("ae_W2", "ad_W1", "ad_W2", "se_W2", "sd_W1"):
            w_small[nm] = wp.tile([128, t[nm].shape[1]], BF16, name=f"w_{nm}")
            nc.sync.dma_start(out=w_small[nm][:], in_=t[nm][:])
        biases = {}
        for nm in ("ae_b1", "ae_b2", "ad_b1", "se_b1", "se_b2", "sd_b1"):
            biases[nm] = wp.tile([128, 1], F32, name=f"b_{nm}")
            nc.sync.dma_start(out=biases[nm][:], in_=t[nm][:])
        b_ad2 = wp.tile([1, XD], BF16)
        nc.sync.dma_start(out=b_ad2[:], in_=t["ad_b2r"][:])
        b_sd2 = wp.tile([1, SD], BF16)
        nc.sync.dma_start(out=b_sd2[:], in_=t["sd_b2r"][:])
        cinv_r = wp.tile([1, NL], F32)
        nc.sync.dma_start(out=cinv_r[:], in_=t["cinv"][:])
        mask_r = wp.tile([1, NL], F32)
        nc.sync.dma_start(out=mask_r[:], in_=t["mask"][:])
        ident = wp.tile([128, 128], F32)
        make_identity(nc, ident[:])
        ones = wp.tile([128, 1], F32)
        nc.gpsimd.memset(ones[:], 1.0)
        neg2 = wp.tile([128, 1], F32)
        nc.gpsimd.memset(neg2[:], -2.0)
        ones1r = wp.tile([1, 128], BF16)
        nc.gpsimd.memset(ones1r[:], 1.0)

        # resident activations (transposed space: [hid, nodes])
        hT_f32 = {"a": wp.tile([128, NL], F32, name="haT_f32"),
                  "s": wp.tile([128, NL], F32, name="hsT_f32")}
        hT_bf = {"a": wp.tile([128, NL], BF16, name="haT_bf"),
                 "s": wp.tile([128, NL], BF16, name="hsT_bf")}

        w_sd2 = wp.tile([128, SD], BF16)
        tbl_in = dram.tile([NL, TD], BF16)
        tbl_out = dram.tile([N, TD], BF16, addr_space="Shared")

        # ---------------- x-encoder ----------------
        with nc.named_scope("xenc"):
            xT_t = io.tile([128, 2 * NL], BF16, name="xT_t", bufs=1)
            for k in range(2):
                nc.sync.dma_start(out=xT_t[:, k * NL:(k + 1) * NL],
                                  in_=t["xT"][k * 128:(k + 1) * 128, :])
            h1aT = psA.tile([128, NL], F32, tag="psA")
            for k in range(2):
                for j in range(2):
                    nc.tensor.matmul(h1aT[:, j * 512:(j + 1) * 512],
                                     w_ae1[:, k * H:(k + 1) * H],
                                     xT_t[:, k * NL + j * 512: k * NL + (j + 1) * 512],
                                     start=(k == 0), stop=(k == 1))
            h1aT_bf = io.tile([128, NL], BF16, name="h1aT_bf", bufs=1)
            nc.scalar.activation(h1aT_bf[:], h1aT[:], Relu, bias=biases["ae_b1"][:])
            haT_ps = psA.tile([128, NL], F32, tag="psA")
            for j in range(2):
                nc.tensor.matmul(haT_ps[:, j * 512:(j + 1) * 512], w_small["ae_W2"][:],
                                 h1aT_bf[:, j * 512:(j + 1) * 512],
                                 start=True, stop=True)
            nc.scalar.activation(hT_f32["a"][:], haT_ps[:], Ident,
                                 bias=biases["ae_b2"][:])
            nc.vector.tensor_copy(hT_bf["a"][:], hT_f32["a"][:])

        # ---------------- s-encoder ----------------
        with nc.named_scope("senc"):
            h1sT = psA.tile([128, NL], F32, tag="psA")
            for k in range(SC):
                sT_t = io.tile([128, NL], BF16, name="sT_t", tag="sT_t", bufs=4)
                nc.sync.dma_start(out=sT_t[:], in_=t["sT"][k * 128:(k + 1) * 128, :])
                for j in range(2):
                    nc.tensor.matmul(h1sT[:, j * 512:(j + 1) * 512],
                                     w_se1[:, k * H:(k + 1) * H],
                                     sT_t[:, j * 512:(j + 1) * 512],
                                     start=(k == 0), stop=(k == SC - 1))
            h1sT_bf = io.tile([128, NL], BF16, name="h1sT_bf", bufs=1)
            nc.scalar.activation(h1sT_bf[:], h1sT[:], Relu, bias=biases["se_b1"][:])
            hsT_ps = psA.tile([128, NL], F32, tag="psA")
            for j in range(2):
                nc.tensor.matmul(hsT_ps[:, j * 512:(j + 1) * 512], w_small["se_W2"][:],
                                 h1sT_bf[:, j * 512:(j + 1) * 512],
                                 start=True, stop=True)
            nc.scalar.activation(hT_f32["s"][:], hsT_ps[:], Ident,
                                 bias=biases["se_b2"][:])
            nc.vector.tensor_copy(hT_bf["s"][:], hT_f32["s"][:])

        # decoder-phase weights, loaded during the s-encoder's DMA window
        nc.sync.dma_start(out=w_sd2[:], in_=t["sd_W2"][:])

        # ------- transposes to node-major; write h outputs + allgather table ----
        with nc.named_scope("tbl"):
            for tt in range(NL // 128):
                blk = io.tile([128, TD], BF16, name="blk", tag="blk", bufs=2)
                for ab, (col0, out_nm) in (("a", (0, "o_ha")), ("s", (H, "o_hs"))):
                    tr = psTr.tile([128, 128], F32, tag="tr")
                    nc.tensor.transpose(tr[:], hT_f32[ab][:, tt * 128:(tt + 1) * 128],
                                        ident[:])
                    nm_f32 = io.tile([128, 128], F32, name="nm_f32", tag="nm_f32",
                                     bufs=2)
                    nc.vector.tensor_copy(nm_f32[:], tr[:])
                    nc.sync.dma_start(out=t[out_nm][tt * 128:(tt + 1) * 128, :],
                                      in_=nm_f32[:])
                    nc.vector.tensor_copy(blk[:, col0:col0 + H], tr[:])
                    # hsq column (f32 square-accumulate, cast to bf16)
                    sq = io.tile([128, 128], F32, name="sq", tag="sq", bufs=2)
                    hsq_c = io.tile([128, 1], F32, name="hsq_c", tag="hsq_c", bufs=2)
                    nc.scalar.activation(sq[:], nm_f32[:], Square,
                                         accum_out=hsq_c[:])
                    nc.vector.tensor_copy(blk[:, 2 * H + (0 if ab == "a" else 1):
                                              2 * H + (1 if ab == "a" else 2)],
                                          hsq_c[:])
                nc.sync.dma_start(out=tbl_in[tt * 128:(tt + 1) * 128, :], in_=blk[:])

            nc.gpsimd.collective_compute(
                "AllGather", mybir.AluOpType.bypass,
                replica_groups=[list(range(NCORES))],
                ins=[tbl_in.opt()], outs=[tbl_out.opt()],
            )
        ctx1.close()  # free phase-1 PSUM (psA/psTr) for phase-2 pools

        psD = ctx.enter_context(tc.tile_pool(name="psD", bufs=1, space="PSUM"))
        psB = ctx.enter_context(tc.tile_pool(name="psB", bufs=1, space="PSUM"))

        def evict(i, out_ap, in_ap):
            if i % 2 == 0:
                nc.scalar.copy(out_ap, in_ap)
            else:
                nc.vector.tensor_copy(out_ap, in_ap)

        # ---------------- x-decoder ----------------
        with nc.named_scope("xdec"):
            g1aT_bf = io.tile([128, NL], BF16, name="g1aT_bf", bufs=1)
            for j in range(2):
                g1 = psD.tile([128, 512], F32, tag="dec", bufs=3)
                nc.tensor.matmul(g1[:], w_small["ad_W1"][:],
                                 hT_bf["a"][:, j * 512:(j + 1) * 512],
                                 start=True, stop=True)
                nc.scalar.activation(g1aT_bf[:, j * 512:(j + 1) * 512], g1[:], Relu,
                                     bias=biases["ad_b1"][:])
            for tt in range(NL // 128):
                xo = psD.tile([128, XD], F32, tag="dec", bufs=3)
                nc.tensor.matmul(xo[:], g1aT_bf[:, tt * 128:(tt + 1) * 128],
                                 w_small["ad_W2"][:], start=True, stop=False)
                nc.tensor.matmul(xo[:], ones1r[:], b_ad2[:], start=False, stop=True)
                xo_sb = io.tile([128, XD], F32, name="xo_sb", tag="xo_sb", bufs=2)
                evict(tt, xo_sb[:], xo[:])
                nc.sync.dma_start(out=t["o_x"][tt * 128:(tt + 1) * 128, :],
                                  in_=xo_sb[:])

        # ---------------- s-decoder ----------------
        with nc.named_scope("sdec"):
            g1sT_bf = io.tile([128, NL], BF16, name="g1sT_bf", bufs=1)
            for j in range(2):
                g1 = psD.tile([128, 512], F32, tag="dec", bufs=3)
                nc.tensor.matmul(g1[:], w_small["sd_W1"][:],
                                 hT_bf["s"][:, j * 512:(j + 1) * 512],
                                 start=True, stop=True)
                nc.scalar.activation(g1sT_bf[:, j * 512:(j + 1) * 512], g1[:], Relu,
                                     bias=biases["sd_b1"][:])
            for tt in range(NL // 128):
                for j in range(SD // 512):
                    so = psD.tile([128, 512], F32, tag="dec", bufs=3)
                    nc.tensor.matmul(so[:], g1sT_bf[:, tt * 128:(tt + 1) * 128],
                                     w_sd2[:, j * 512:(j + 1) * 512],
                                     start=True, stop=False)
                    nc.tensor.matmul(so[:], ones1r[:], b_sd2[:, j * 512:(j + 1) * 512],
                                     start=False, stop=True)
                    so_sb = io.tile([128, 512], BF16, name="so_sb", tag="so_sb",
                                    bufs=4)
                    evict(tt * 16 + j, so_sb[:], so[:])
                    nc.sync.dma_start(
                        out=t["o_s"][tt * 128:(tt + 1) * 128, j * 512:(j + 1) * 512],
                        in_=so_sb[:])

        # ------- B-matmul: [m_a | m_s | q] = table^T-contracted with BT -------
        # two sequential 512-wide dst passes; accumulators evicted to SBUF
        with nc.named_scope("bmm"):
            tblp = ctx.enter_context(tc.tile_pool(name="tblp", bufs=1))
            tbl_sb = tblp.tile([128, KC * TD], BF16)
            for k in range(KC):
                nc.sync.dma_start(out=tbl_sb[:, k * TD:(k + 1) * TD],
                                  in_=tbl_out[k * 128:(k + 1) * 128, :])
            dna_sb = {}
            for ab in ("a", "s"):
                dna_sb[ab] = io.tile([1, NL], F32, name=f"dna_{ab}",
                                     tag=f"dna_{ab}", bufs=1)

            for j in range(2):
                sl = slice(j * 512, (j + 1) * 512)
                m_a = psB.tile([128, 512], F32, tag="m_a")
                m_s = psB.tile([128, 512], F32, tag="m_s")
                m_q = psB.tile([2, 512], F32, tag="m_q")
                for k in range(KC):
                    bt_t = io.tile([128, 512], BF16, name="bt_t", tag="bt_t",
                                   bufs=16)
                    nc.sync.dma_start(out=bt_t[:],
                                      in_=t["BT"][k * 128:(k + 1) * 128, sl])
                    st = dict(start=(k == 0), stop=(k == KC - 1))
                    nc.tensor.matmul(m_a[:], tbl_sb[:, k * TD:k * TD + H],
                                     bt_t[:], **st)
                    nc.tensor.matmul(m_s[:], tbl_sb[:, k * TD + H:k * TD + 2 * H],
                                     bt_t[:], **st)
                    nc.tensor.matmul(m_q[:], tbl_sb[:, k * TD + 2 * H:k * TD + TD],
                                     bt_t[:], **st)
                # evict accumulators to SBUF, freeing PSUM for the next pass
                m_sb = {}
                for nm, ps, i in (("a", m_a, 0), ("s", m_s, 1)):
                    m_sb[nm] = io.tile([128, 512], F32, name=f"m_sb_{nm}",
                                       tag=f"m_sb_{nm}", bufs=2)
                    evict(i, m_sb[nm][:], ps[:])
                q_sb = io.tile([2, 512], F32, name="q_sb", tag="q_sb", bufs=2)
                nc.vector.tensor_copy(q_sb[:], m_q[:])

                # ---- dna / dns for this 512-node chunk ----
                # dna = mask * (hsq + cinv*(q - 2*dot)); q - 2*dot built on PE
                for ab, qrow in (("a", 0), ("s", 1)):
                    prod = io.tile([128, 512], F32, name="prod", tag="prod", bufs=2)
                    nc.vector.tensor_tensor(prod[:], m_sb[ab][:],
                                            hT_f32[ab][:, sl], mult)
                    dotq = psD.tile([1, 512], F32, tag="dot", bufs=2)
                    nc.tensor.matmul(dotq[:], ident[0:2, qrow:qrow + 1], q_sb[:],
                                     start=True, stop=False)
                    nc.tensor.matmul(dotq[:], neg2[:], prod[:],
                                     start=False, stop=True)
                    sqT = io.tile([128, 512], F32, name="sqT", tag="sqT", bufs=2)
                    nc.scalar.activation(sqT[:], hT_f32[ab][:, sl], Square)
                    hsq_r = psD.tile([1, 512], F32, tag="dot", bufs=2)
                    nc.tensor.matmul(hsq_r[:], ones[:], sqT[:],
                                     start=True, stop=True)
                    tmp = io.tile([1, 512], F32, name="tmp", tag="tmp", bufs=2)
                    nc.vector.tensor_tensor(tmp[:], dotq[:], cinv_r[:, sl], mult)
                    nc.vector.tensor_tensor(tmp[:], tmp[:], hsq_r[:], add)
                    nc.vector.tensor_tensor(dna_sb[ab][:, sl], tmp[:],
                                            mask_r[:, sl], mult)
            nc.sync.dma_start(out=t["o_dna"][:], in_=dna_sb["a"][:])
            nc.sync.dma_start(out=t["o_dns"][:], in_=dna_sb["s"][:])


def _prep_inputs(x, s, edge_index,
                 ae_W1, ae_b1, ae_W2, ae_b2,
                 ad_W1, ad_b1, ad_W2, ad_b2,
                 se_W1, se_b1, se_W2, se_b2,
                 sd_W1, sd_b1, sd_W2, sd_b2):
    bf = ml_dtypes.bfloat16
    src = np.asarray(edge_index[0], dtype=np.int64)
    dst = np.asarray(edge_index[1], dtype=np.int64)
    cnt = np.bincount(dst, minlength=N).astype(np.float32)
    cinv = (1.0 / np.maximum(cnt, 1.0)).astype(np.float32)
    mask = (cnt > 0).astype(np.float32)
    # dense count matrix B^T[src, dst]
    BT_full = np.bincount(src * N + dst, minlength=N * N).reshape(N, N)
    BT_full = BT_full.astype(bf)

    shared = dict(
        ae_W1=ae_W1.astype(bf), ae_W2=ae_W2.astype(bf),
        ad_W1=ad_W1.astype(bf), ad_W2=ad_W2.astype(bf),
        se_W1=se_W1.astype(bf), se_W2=se_W2.astype(bf),
        sd_W1=sd_W1.astype(bf), sd_W2=sd_W2.astype(bf),
        ae_b1=ae_b1.reshape(H, 1).astype(np.float32),
        ae_b2=ae_b2.reshape(H, 1).astype(np.float32),
        ad_b1=ad_b1.reshape(H, 1).astype(np.float32),
        se_b1=se_b1.reshape(H, 1).astype(np.float32),
        se_b2=se_b2.reshape(H, 1).astype(np.float32),
        sd_b1=sd_b1.reshape(H, 1).astype(np.float32),
        ad_b2r=ad_b2.reshape(1, XD).astype(bf),
        sd_b2r=sd_b2.reshape(1, SD).astype(bf),
    )
    x = np.asarray(x, dtype=np.float32)
    s = np.asarray(s, dtype=np.float32)
    in_maps = []
    for c in range(NCORES):
        rows = slice(c * NL, (c + 1) * NL)
        m = dict(shared)
        m["xT"] = np.ascontiguousarray(x[rows].T).astype(bf)
        m["sT"] = np.ascontiguousarray(s[rows].T).astype(bf)
        m["BT"] = np.ascontiguousarray(BT_full[:, rows])
        m["cinv"] = cinv[rows].reshape(1, NL)
        m["mask"] = mask[rows].reshape(1, NL)
        in_maps.append(m)
    return in_maps


def _run(in_maps, trace=False):
    if "nc" not in _CACHE:
        _CACHE["nc"] = _build_program()
    nc = _CACHE["nc"]
    return run_bass_kernel_spmd(nc, in_maps, core_ids=list(range(NCORES)),
                                trace=trace)


def _assemble(results):
    x_ = np.concatenate([results[c]["o_x"] for c in range(NCORES)], axis=0)
    s_ = np.concatenate([results[c]["o_s"].astype(np.float32)
                         for c in range(NCORES)], axis=0)
    h_a = np.concatenate([results[c]["o_ha"] for c in range(NCORES)], axis=0)
    h_s = np.concatenate([results[c]["o_hs"] for c in range(NCORES)], axis=0)
    dna = np.concatenate([results[c]["o_dna"].reshape(NL)
                          for c in range(NCORES)], axis=0)
    dns = np.concatenate([results[c]["o_dns"].reshape(NL)
                          for c in range(NCORES)], axis=0)
    return (x_.astype(np.float32), s_, h_a.astype(np.float32),
            h_s.astype(np.float32), dna.astype(np.float32),
            dns.astype(np.float32))


def kernel(**inputs):
    in_maps = _prep_inputs(**inputs)
    res = _run(in_maps, trace=False)
    return _assemble(res.results)


def kernel_traced(**inputs):
    """Like kernel(), but also returns neuron-profile exec time in ns."""
    in_maps = _prep_inputs(**inputs)
    res = _run(in_maps, trace=True)
    return _assemble(res.results), res.exec_time_ns


# revision 8
# speedup vs baseline: 1.3430x; 1.3430x over previous
"""Trainium2 Bass kernel: GNN message passing (2x encoder/decoder MLP + NeighDiff).

Sharding: nodes row-sharded across 8 NeuronCores (1024 nodes/core).
The segment-mean NeighDiff is computed via the algebraic expansion
    dna_i = mask_i * (||h_i||^2 + (q_i - 2 h_i . m_i) / c_i)
with [m | q] = B @ [h | hsq], where B is the dense [dst, src] edge-count
matrix (built host-side from edge_index as part of sharding prep) and the
full node embedding table is AllGathered on-device in bf16.

All large tensors are staged host-side in partition-major packed layouts so
every DMA moves large contiguous descriptors.
"""

import numpy as np
import ml_dtypes

import concourse.bass as bass
import concourse.mybir as mybir
import concourse.tile as tile
from concourse import bacc
from concourse.bass_utils import run_bass_kernel_spmd
from concourse.masks import make_identity

BF16 = mybir.dt.bfloat16
F32 = mybir.dt.float32

NCORES = 8
N = 8192          # nodes
NL = N // NCORES  # nodes per core (1024)
XD = 256          # x feature dim
H = 128           # hidden dim
SD = 8192         # s feature dim
TD = 2 * H + 2    # table cols: h_a | h_s | hsq_a | hsq_s
KC = N // 128     # k-chunks over source nodes (64)
SC = SD // 128    # k-chunks over s columns (64)
NT = NL // 128    # node tiles per core (8)

_CACHE = {}


def _pk(a):
    """[G*128, C] -> [128, G*C] partition-major packing."""
    g = a.shape[0] // 128
    return np.ascontiguousarray(
        a.reshape(g, 128, a.shape[1]).transpose(1, 0, 2).reshape(128, -1))


def _upk(a, c):
    """[128, G*C] -> [G*128, C] unpack."""
    g = a.shape[1] // c
    return np.ascontiguousarray(
        a.reshape(128, g, c).transpose(1, 0, 2).reshape(g * 128, c))


def _build_program():
    nc = bacc.Bacc("TRN2", target_bir_lowering=False, debug=False,
                   num_devices=NCORES)

    def din(name, shape, dt):
        return nc.dram_tensor(name, shape, dt, kind="ExternalInput").ap()

    def dout(name, shape, dt):
        return nc.dram_tensor(name, shape, dt, kind="ExternalOutput").ap()

    # per-core sharded inputs (packed layouts)
    xT = din("xT", [128, 2 * NL], BF16)       # x rows transposed, packed
    sT = din("sT", [128, SC * NL], BF16)      # s rows transposed, packed by k-chunk
    BT = din("BT", [128, 2 * KC * 512], BF16)  # count matrix, packed [j, k, 512]
    cinv = din("cinv", [1, NL], F32)
    mask = din("mask", [1, NL], F32)
    se_W1 = din("se_W1", [128, SC * H], BF16)  # packed k-chunks
    ae_W1 = din("ae_W1", [128, 2 * H], BF16)
    sd_W2 = din("sd_W2", [128, SD], BF16)
    # small weights packed: ae_W2 | ad_W1 | ad_W2 | se_W2 | sd_W1
    wpack = din("wpack", [128, 768], BF16)
    bpack = din("bpack", [128, 6], F32)       # ae_b1|ae_b2|ad_b1|se_b1|se_b2|sd_b1
    ad_b2b = din("ad_b2b", [128, XD], F32)    # broadcast decoder output biases
    sd_b2b = din("sd_b2b", [128, SD], BF16)

    # per-core outputs (packed layouts, unpacked on host)
    o_x = dout("o_x", [128, NT * XD], BF16)
    o_s = dout("o_s", [128, NT * SD], BF16)
    o_ha = dout("o_ha", [128, NT * H], F32)
    o_hs = dout("o_hs", [128, NT * H], F32)
    o_dna = dout("o_dna", [1, NL], F32)
    o_dns = dout("o_dns", [1, NL], F32)

    with tile.TileContext(nc) as tc:
        _emit(nc, tc, locals())
    nc.compile()
    return nc


def _emit(nc, tc, t):
    Relu = mybir.ActivationFunctionType.Relu
    Ident = mybir.ActivationFunctionType.Identity
    Square = mybir.ActivationFunctionType.Square
    add = mybir.AluOpType.add
    mult = mybir.AluOpType.mult

    import contextlib
    ctx = contextlib.ExitStack()
    ctx1 = contextlib.ExitStack()  # phase-1 pools, closed before phase 2
    with ctx:
        wp = ctx.enter_context(tc.tile_pool(name="wp", bufs=1))
        io = ctx.enter_context(tc.tile_pool(name="io", bufs=2))
        dram = ctx.enter_context(tc.tile_pool(name="dram", bufs=1, space="DRAM"))

        # --- phase-1 critical weights ---
        w_se1 = wp.tile([128, SC * H], BF16)
        nc.sync.dma_start(out=w_se1[:], in_=t["se_W1"][:])
        w_ae1 = wp.tile([128, 2 * H], BF16)
        nc.sync.dma_start(out=w_ae1[:], in_=t["ae_W1"][:])
        wpk = wp.tile([128, 768], BF16)
        nc.sync.dma_start(out=wpk[:], in_=t["wpack"][:])
        W = {"ae_W2": wpk[:, 0:128], "ad_W1": wpk[:, 128:256],
             "ad_W2": wpk[:, 256:512], "se_W2": wpk[:, 512:640],
             "sd_W1": wpk[:, 640:768]}
        bpk = wp.tile([128, 6], F32)
        nc.sync.dma_start(out=bpk[:], in_=t["bpack"][:])
        B_ = {nm: bpk[:, i:i + 1] for i, nm in enumerate(
            ("ae_b1", "ae_b2", "ad_b1", "se_b1", "se_b2", "sd_b1"))}
        cinv_r = wp.tile([1, NL], F32)
        nc.sync.dma_start(out=cinv_r[:], in_=t["cinv"][:])
        mask_r = wp.tile([1, NL], F32)
        nc.sync.dma_start(out=mask_r[:], in_=t["mask"][:])
        ident = wp.tile([128, 128], F32)
        make_identity(nc, ident[:])
        ones = wp.tile([128, 1], F32)
        nc.gpsimd.memset(ones[:], 1.0)
        neg2 = wp.tile([128, 1], F32)
        nc.gpsimd.memset(neg2[:], -2.0)

        hT_bf = {"a": wp.tile([128, NL], BF16, name="haT_bf"),
                 "s": wp.tile([128, NL], BF16, name="hsT_bf")}
        ha_asm = {"a": wp.tile([128, NT * H], F32, name="ha_asm"),
                  "s": wp.tile([128, NT * H], F32, name="hs_asm")}
        x_asm = wp.tile([128, NT * XD], BF16)
        tbl_asm = wp.tile([128, NT * TD], BF16)

        tbl_in = dram.tile([128, NT * TD], BF16)
        tbl_out = dram.tile([NCORES * 128, NT * TD], BF16, addr_space="Shared")

        ioH = ctx1.enter_context(tc.tile_pool(name="ioH", bufs=1))
        hT_f32 = {"a": ioH.tile([128, NL], F32, name="haT_f32"),
                  "s": ioH.tile([128, NL], F32, name="hsT_f32")}
        ioS = ctx1.enter_context(tc.tile_pool(name="ioS", bufs=3))
        psA = ctx1.enter_context(tc.tile_pool(name="psA", bufs=2, space="PSUM"))
        psTr = ctx1.enter_context(tc.tile_pool(name="psTr", bufs=2, space="PSUM"))

        # ---------------- x-encoder ----------------
        with nc.named_scope("xenc"):
            xT_t = io.tile([128, 2 * NL], BF16, name="xT_t", bufs=1)
            nc.sync.dma_start(out=xT_t[:], in_=t["xT"][:])
            h1aT = psA.tile([128, NL], F32, tag="psA")
            for k in range(2):
                for j in range(2):
                    nc.tensor.matmul(h1aT[:, j * 512:(j + 1) * 512],
                                     w_ae1[:, k * H:(k + 1) * H],
                                     xT_t[:, k * NL + j * 512: k * NL + (j + 1) * 512],
                                     start=(k == 0), stop=(k == 1))
            h1aT_bf = io.tile([128, NL], BF16, name="h1aT_bf", bufs=1)
            nc.scalar.activation(h1aT_bf[:], h1aT[:], Relu, bias=B_["ae_b1"])
            haT_ps = psA.tile([128, NL], F32, tag="psA")
            for j in range(2):
                nc.tensor.matmul(haT_ps[:, j * 512:(j + 1) * 512], W["ae_W2"],
                                 h1aT_bf[:, j * 512:(j + 1) * 512],
                                 start=True, stop=True)
            nc.scalar.activation(hT_f32["a"][:], haT_ps[:], Ident, bias=B_["ae_b2"])
            nc.vector.tensor_copy(hT_bf["a"][:], hT_f32["a"][:])

        # ---------------- s-encoder ----------------
        with nc.named_scope("senc"):
            h1sT = psA.tile([128, NL], F32, tag="psA")
            for kg in range(SC // 4):          # 16 loads of 4 k-chunks each
                sT_t = ioS.tile([128, 4 * NL], BF16, name="sT_t", tag="sT_t")
                nc.sync.dma_start(out=sT_t[:],
                                  in_=t["sT"][:, kg * 4 * NL:(kg + 1) * 4 * NL])
                for kk in range(4):
                    k = kg * 4 + kk
                    for j in range(2):
                        nc.tensor.matmul(h1sT[:, j * 512:(j + 1) * 512],
                                         w_se1[:, k * H:(k + 1) * H],
                                         sT_t[:, kk * NL + j * 512:
                                              kk * NL + (j + 1) * 512],
                                         start=(k == 0), stop=(k == SC - 1))
            h1sT_bf = io.tile([128, NL], BF16, name="h1sT_bf", bufs=1)
            nc.scalar.activation(h1sT_bf[:], h1sT[:], Relu, bias=B_["se_b1"])
            hsT_ps = psA.tile([128, NL], F32, tag="psA")
            for j in range(2):
                nc.tensor.matmul(hsT_ps[:, j * 512:(j + 1) * 512], W["se_W2"],
                                 h1sT_bf[:, j * 512:(j + 1) * 512],
                                 start=True, stop=True)
            nc.scalar.activation(hT_f32["s"][:], hsT_ps[:], Ident, bias=B_["se_b2"])
            nc.vector.tensor_copy(hT_bf["s"][:], hT_f32["s"][:])

        # decoder-phase weights (behind the s-encoder stream in queue order)
        w_sd2 = wp.tile([128, SD], BF16)
        nc.sync.dma_start(out=w_sd2[:], in_=t["sd_W2"][:])
        b_ad2 = wp.tile([128, XD], F32)
        nc.sync.dma_start(out=b_ad2[:], in_=t["ad_b2b"][:])
        b_sd2 = wp.tile([128, SD], BF16)
        nc.sync.dma_start(out=b_sd2[:], in_=t["sd_b2b"][:])

        # ------- transposes to node-major; h outputs + allgather table ----
        with nc.named_scope("tbl"):
            for tt in range(NT):
                for ab, col0 in (("a", 0), ("s", H)):
                    tr = psTr.tile([128, 128], F32, tag="tr")
                    nc.tensor.transpose(tr[:], hT_f32[ab][:, tt * 128:(tt + 1) * 128],
                                        ident[:])
                    nc.vector.tensor_copy(ha_asm[ab][:, tt * H:(tt + 1) * H], tr[:])
                    nc.vector.tensor_copy(tbl_asm[:, tt * TD + col0:
                                                  tt * TD + col0 + H], tr[:])
                    sq = io.tile([128, 128], F32, name="sq", tag="sq", bufs=2)
                    hsq_c = io.tile([128, 1], F32, name="hsq_c", tag="hsq_c", bufs=2)
                    nc.scalar.activation(sq[:], ha_asm[ab][:, tt * H:(tt + 1) * H],
                                         Square, accum_out=hsq_c[:])
                    o = tt * TD + 2 * H + (0 if ab == "a" else 1)
                    nc.vector.tensor_copy(tbl_asm[:, o:o + 1], hsq_c[:])
            nc.sync.dma_start(out=t["o_ha"][:], in_=ha_asm["a"][:])
            nc.sync.dma_start(out=t["o_hs"][:], in_=ha_asm["s"][:])
            nc.sync.dma_start(out=tbl_in[:], in_=tbl_asm[:])
            nc.gpsimd.collective_compute(
                "AllGather", mybir.AluOpType.bypass,
                replica_groups=[list(range(NCORES))],
                ins=[tbl_in.opt()], outs=[tbl_out.opt()],
            )
        ctx1.close()  # free phase-1 SBUF/PSUM pools

        psD = ctx.enter_context(tc.tile_pool(name="psD", bufs=1, space="PSUM"))
        psB = ctx.enter_context(tc.tile_pool(name="psB", bufs=1, space="PSUM"))
        ioB = ctx.enter_context(tc.tile_pool(name="ioB", bufs=3))

        # ---------------- x-decoder ----------------
        with nc.named_scope("xdec"):
            g1aT_bf = io.tile([128, NL], BF16, name="g1aT_bf", bufs=1)
            for j in range(2):
                g1 = psD.tile([128, 512], F32, tag="dec", bufs=3)
                nc.tensor.matmul(g1[:], W["ad_W1"],
                                 hT_bf["a"][:, j * 512:(j + 1) * 512],
                                 start=True, stop=True)
                nc.scalar.activation(g1aT_bf[:, j * 512:(j + 1) * 512], g1[:], Relu,
                                     bias=B_["ad_b1"])
            for tt in range(NT):
                xo = psD.tile([128, XD], F32, tag="dec", bufs=3)
                nc.tensor.matmul(xo[:], g1aT_bf[:, tt * 128:(tt + 1) * 128],
                                 W["ad_W2"], start=True, stop=True)
                nc.vector.tensor_tensor(x_asm[:, tt * XD:(tt + 1) * XD], xo[:],
                                        b_ad2[:], add)
            nc.sync.dma_start(out=t["o_x"][:], in_=x_asm[:])

        # ---------------- s-decoder ----------------
        with nc.named_scope("sdec"):
            g1sT_bf = io.tile([128, NL], BF16, name="g1sT_bf", bufs=1)
            for j in range(2):
                g1 = psD.tile([128, 512], F32, tag="dec", bufs=3)
                nc.tensor.matmul(g1[:], W["sd_W1"],
                                 hT_bf["s"][:, j * 512:(j + 1) * 512],
                                 start=True, stop=True)
                nc.scalar.activation(g1sT_bf[:, j * 512:(j + 1) * 512], g1[:], Relu,
                                     bias=B_["sd_b1"])
            for tt in range(NT):
                for half in range(2):
                    s_asm = ioB.tile([128, SD // 2], BF16, name="s_asm", tag="s_asm",
                                     bufs=3)
                    for jj in range(8):
                        j = half * 8 + jj
                        so = psD.tile([128, 512], F32, tag="dec", bufs=3)
                        nc.tensor.matmul(so[:], g1sT_bf[:, tt * 128:(tt + 1) * 128],
                                         w_sd2[:, j * 512:(j + 1) * 512],
                                         start=True, stop=True)
                        nc.vector.tensor_tensor(s_asm[:, jj * 512:(jj + 1) * 512],
                                                so[:], b_sd2[:, j * 512:(j + 1) * 512],
                                                add)
                    nc.sync.dma_start(
                        out=t["o_s"][:, tt * SD + half * (SD // 2):
                                     tt * SD + (half + 1) * (SD // 2)],
                        in_=s_asm[:])

        # ------- B-matmul: [m_a | m_s | q] = table^T-contracted with BT -------
        with nc.named_scope("bmm"):
            tblp = ctx.enter_context(tc.tile_pool(name="tblp", bufs=1))
            tbl_sb = tblp.tile([128, KC * TD], BF16)
            for r in range(NCORES):
                nc.sync.dma_start(out=tbl_sb[:, r * NT * TD:(r + 1) * NT * TD],
                                  in_=tbl_out[r * 128:(r + 1) * 128, :])
            dna_sb = {}
            for ab in ("a", "s"):
                dna_sb[ab] = io.tile([1, NL], F32, name=f"dna_{ab}",
                                     tag=f"dna_{ab}", bufs=1)

            for j in range(2):
                sl = slice(j * 512, (j + 1) * 512)
                m_a = psB.tile([128, 512], F32, tag="m_a")
                m_s = psB.tile([128, 512], F32, tag="m_s")
                m_q = psB.tile([2, 512], F32, tag="m_q")
                for kg in range(KC // 4):     # 16 loads of 4 k-chunks per pass
                    bt_t = ioB.tile([128, 4 * 512], BF16, name="bt_t", tag="bt_t",
                                    bufs=4)
                    base = j * KC * 512 + kg * 4 * 512
                    nc.sync.dma_start(out=bt_t[:],
                                      in_=t["BT"][:, base:base + 4 * 512])
                    for kk in range(4):
                        k = kg * 4 + kk
                        rhs = bt_t[:, kk * 512:(kk + 1) * 512]
                        st = dict(start=(k == 0), stop=(k == KC - 1))
                        nc.tensor.matmul(m_a[:], tbl_sb[:, k * TD:k * TD + H],
                                         rhs, **st)
                        nc.tensor.matmul(m_s[:], tbl_sb[:, k * TD + H:k * TD + 2 * H],
                                         rhs, **st)
                        nc.tensor.matmul(m_q[:], tbl_sb[:, k * TD + 2 * H:
                                                        k * TD + TD], rhs, **st)
                q_sb = io.tile([2, 512], F32, name="q_sb", tag="q_sb", bufs=2)
                nc.vector.tensor_copy(q_sb[:], m_q[:])

                # ---- dna / dns for this 512-node chunk ----
                # dna = mask * (hsq + cinv*(q - 2*dot)); q - 2*dot built on PE
                for ab, (qrow, m_ps) in (("a", (0, m_a)), ("s", (1, m_s))):
                    prod = io.tile([128, 512], F32, name="prod", tag="prod", bufs=2)
                    nc.vector.tensor_tensor(prod[:], m_ps[:], hT_bf[ab][:, sl], mult)
                    dotq = psD.tile([1, 512], F32, tag="dot", bufs=2)
                    nc.tensor.matmul(dotq[:], ident[0:2, qrow:qrow + 1], q_sb[:],
                                     start=True, stop=False)
                    nc.tensor.matmul(dotq[:], neg2[:], prod[:],
                                     start=False, stop=True)
                    sqT = io.tile([128, 512], F32, name="sqT", tag="sqT", bufs=2)
                    nc.scalar.activation(sqT[:], hT_bf[ab][:, sl], Square)
                    hsq_r = psD.tile([1, 512], F32, tag="dot", bufs=2)
                    nc.tensor.matmul(hsq_r[:], ones[:], sqT[:],
                                     start=True, stop=True)
                    tmp = io.tile([1, 512], F32, name="tmp", tag="tmp", bufs=2)
                    nc.vector.tensor_tensor(tmp[:], dotq[:], cinv_r[:, sl], mult)
                    nc.vector.tensor_tensor(tmp[:], tmp[:], hsq_r[:], add)
                    nc.vector.tensor_tensor(dna_sb[ab][:, sl], tmp[:],
                                            mask_r[:, sl], mult)
            nc.sync.dma_start(out=t["o_dna"][:], in_=dna_sb["a"][:])
            nc.sync.dma_start(out=t["o_dns"][:], in_=dna_sb["s"][:])


def _prep_inputs(x, s, edge_index,
                 ae_W1, ae_b1, ae_W2, ae_b2,
                 ad_W1, ad_b1, ad_W2, ad_b2,
                 se_W1, se_b1, se_W2, se_b2,
                 sd_W1, sd_b1, sd_W2, sd_b2):
    bf = ml_dtypes.bfloat16
    src = np.asarray(edge_index[0], dtype=np.int64)
    dst = np.asarray(edge_index[1], dtype=np.int64)
    cnt = np.bincount(dst, minlength=N).astype(np.float32)
    cinv = (1.0 / np.maximum(cnt, 1.0)).astype(np.float32)
    mask = (cnt > 0).astype(np.float32)
    # dense count matrix B^T[src, dst]
    BT_full = np.bincount(src * N + dst, minlength=N * N).reshape(N, N)
    BT_full = BT_full.astype(bf)

    wpack = np.concatenate([ae_W2, ad_W1, ad_W2, se_W2, sd_W1],
                           axis=1).astype(bf)            # [128, 768]
    bpack = np.stack([ae_b1, ae_b2, ad_b1, se_b1, se_b2, sd_b1],
                     axis=1).astype(np.float32)          # [128, 6]
    shared = dict(
        se_W1=_pk(se_W1.astype(bf)),
        ae_W1=_pk(ae_W1.astype(bf)),
        sd_W2=sd_W2.astype(bf),
        wpack=wpack, bpack=bpack,
        ad_b2b=np.ascontiguousarray(np.broadcast_to(
            ad_b2.astype(np.float32), (128, XD))),
        sd_b2b=np.ascontiguousarray(np.broadcast_to(
            sd_b2.astype(bf), (128, SD))),
    )
    x = np.asarray(x, dtype=np.float32)
    s = np.asarray(s, dtype=np.float32)
    in_maps = []
    for c in range(NCORES):
        rows = slice(c * NL, (c + 1) * NL)
        m = dict(shared)
        m["xT"] = _pk(np.ascontiguousarray(x[rows].T).astype(bf))
        m["sT"] = _pk(np.ascontiguousarray(s[rows].T).astype(bf))
        # BT packed as [128, (pass j, k-chunk, 512)]
        btc = np.ascontiguousarray(BT_full[:, rows])     # [8192, 1024]
        btc = btc.reshape(KC, 128, 2, 512).transpose(1, 2, 0, 3)
        m["BT"] = np.ascontiguousarray(btc.reshape(128, 2 * KC * 512))
        m["cinv"] = cinv[rows].reshape(1, NL)
        m["mask"] = mask[rows].reshape(1, NL)
        in_maps.append(m)
    return in_maps


def _run(in_maps, trace=False):
    if "nc" not in _CACHE:
        _CACHE["nc"] = _build_program()
    nc = _CACHE["nc"]
    return run_bass_kernel_spmd(nc, in_maps, core_ids=list(range(NCORES)),
                                trace=trace)


def _assemble(results):
    x_ = np.concatenate([_upk(results[c]["o_x"].astype(np.float32), XD)
                         for c in range(NCORES)], axis=0)
    s_ = np.concatenate([_upk(results[c]["o_s"].astype(np.float32), SD)
                         for c in range(NCORES)], axis=0)
    h_a = np.concatenate([_upk(results[c]["o_ha"], H)
                          for c in range(NCORES)], axis=0)
    h_s = np.concatenate([_upk(results[c]["o_hs"], H)
                          for c in range(NCORES)], axis=0)
    dna = np.concatenate([results[c]["o_dna"].reshape(NL)
                          for c in range(NCORES)], axis=0)
    dns = np.concatenate([results[c]["o_dns"].reshape(NL)
                          for c in range(NCORES)], axis=0)
    return (np.ascontiguousarray(x_, dtype=np.float32), s_,
            np.ascontiguousarray(h_a, dtype=np.float32),
            np.ascontiguousarray(h_s, dtype=np.float32),
            dna.astype(np.float32), dns.astype(np.float32))


def kernel(**inputs):
    in_maps = _prep_inputs(**inputs)
    res = _run(in_maps, trace=False)
    return _assemble(res.results)


def kernel_traced(**inputs):
    """Like kernel(), but also returns neuron-profile exec time in ns."""
    in_maps = _prep_inputs(**inputs)
    res = _run(in_maps, trace=True)
    return _assemble(res.results), res.exec_time_ns


# revision 10
# speedup vs baseline: 1.6431x; 1.2235x over previous
"""Trainium2 Bass kernel: GNN message passing (2x encoder/decoder MLP + NeighDiff).

Sharding: nodes row-sharded across 8 NeuronCores (1024 nodes/core).
The segment-mean NeighDiff is computed via the algebraic expansion
    dna_i = mask_i * (||h_i||^2 + (q_i - 2 h_i . m_i) / c_i)
with [m | q] = B @ [h | hsq], where B is the dense [dst, src] edge-count
matrix (built host-side from edge_index as part of sharding prep) and the
full node embedding table is AllGathered on-device in bf16, split into an
early gather (h_a, hidden under the s-encoder) and a late gather
(h_s | hsq_a | hsq_s, hidden under the s-decoder).

All large tensors are staged host-side in partition-major packed layouts so
every DMA moves large contiguous descriptors. Input loads for the B-matmul
ride the scalar-engine HWDGE queue so the collective-gated loads never
head-of-line-block the store stream on the sync queue.
"""

import numpy as np
import ml_dtypes

import concourse.bass as bass
import concourse.mybir as mybir
import concourse.tile as tile
from concourse import bacc
from concourse.bass_utils import run_bass_kernel_spmd
from concourse.masks import make_identity

BF16 = mybir.dt.bfloat16
F32 = mybir.dt.float32

NCORES = 8
N = 8192          # nodes
NL = N // NCORES  # nodes per core (1024)
XD = 256          # x feature dim
H = 128           # hidden dim
SD = 8192         # s feature dim
TDA = H           # early table cols: h_a
TDS = H + 2       # late table cols: h_s | hsq_a | hsq_s
KC = N // 128     # k-chunks over source nodes (64)
SC = SD // 128    # k-chunks over s columns (64)
NT = NL // 128    # node tiles per core (8)

_CACHE = {}


def _pk(a):
    """[G*128, C] -> [128, G*C] partition-major packing."""
    g = a.shape[0] // 128
    return np.ascontiguousarray(
        a.reshape(g, 128, a.shape[1]).transpose(1, 0, 2).reshape(128, -1))


def _upk(a, c):
    """[128, G*C] -> [G*128, C] unpack."""
    g = a.shape[1] // c
    return np.ascontiguousarray(
        a.reshape(128, g, c).transpose(1, 0, 2).reshape(g * 128, c))


def _build_program():
    nc = bacc.Bacc("TRN2", target_bir_lowering=False, debug=False,
                   num_devices=NCORES)

    def din(name, shape, dt):
        return nc.dram_tensor(name, shape, dt, kind="ExternalInput").ap()

    def dout(name, shape, dt):
        return nc.dram_tensor(name, shape, dt, kind="ExternalOutput").ap()

    # per-core sharded inputs (packed layouts)
    xT = din("xT", [128, 2 * NL], BF16)       # x rows transposed, packed
    sT = din("sT", [128, SC * NL], BF16)      # s rows transposed, packed by k-chunk
    BT = din("BT", [128, 2 * KC * 512], BF16)  # count matrix, packed [j, k, 512]
    cinv = din("cinv", [1, NL], F32)
    mask = din("mask", [1, NL], F32)
    se_W1 = din("se_W1", [128, SC * H], BF16)  # packed k-chunks
    ae_W1 = din("ae_W1", [128, 2 * H], BF16)
    sd_W2 = din("sd_W2", [128, SD], BF16)
    # small weights packed: ae_W2 | ad_W1 | ad_W2 | se_W2 | sd_W1
    wpack = din("wpack", [128, 768], BF16)
    bpack = din("bpack", [128, 6], F32)       # ae_b1|ae_b2|ad_b1|se_b1|se_b2|sd_b1
    ad_b2b = din("ad_b2b", [128, XD], F32)    # broadcast decoder output biases
    sd_b2b = din("sd_b2b", [128, SD], BF16)

    # per-core outputs (packed layouts, unpacked on host)
    o_x = dout("o_x", [128, NT * XD], BF16)
    o_s = dout("o_s", [128, NT * SD], BF16)
    o_ha = dout("o_ha", [128, NT * H], F32)
    o_hs = dout("o_hs", [128, NT * H], F32)
    o_dna = dout("o_dna", [1, NL], F32)
    o_dns = dout("o_dns", [1, NL], F32)

    with tile.TileContext(nc) as tc:
        _emit(nc, tc, locals())
    nc.compile()
    return nc


def _emit(nc, tc, t):
    Relu = mybir.ActivationFunctionType.Relu
    Ident = mybir.ActivationFunctionType.Identity
    Square = mybir.ActivationFunctionType.Square
    add = mybir.AluOpType.add
    mult = mybir.AluOpType.mult

    import contextlib
    ctx = contextlib.ExitStack()
    ctx1 = contextlib.ExitStack()  # phase-1 pools, closed before phase 2
    with ctx:
        wp = ctx.enter_context(tc.tile_pool(name="wp", bufs=1))
        io = ctx.enter_context(tc.tile_pool(name="io", bufs=2))
        dram = ctx.enter_context(tc.tile_pool(name="dram", bufs=1, space="DRAM"))

        # --- phase-1 critical weights ---
        w_se1 = wp.tile([128, SC * H], BF16)
        nc.sync.dma_start(out=w_se1[:], in_=t["se_W1"][:])
        w_ae1 = wp.tile([128, 2 * H], BF16)
        nc.sync.dma_start(out=w_ae1[:], in_=t["ae_W1"][:])
        wpk = wp.tile([128, 768], BF16)
        nc.sync.dma_start(out=wpk[:], in_=t["wpack"][:])
        W = {"ae_W2": wpk[:, 0:128], "ad_W1": wpk[:, 128:256],
             "ad_W2": wpk[:, 256:512], "se_W2": wpk[:, 512:640],
             "sd_W1": wpk[:, 640:768]}
        bpk = wp.tile([128, 6], F32)
        nc.sync.dma_start(out=bpk[:], in_=t["bpack"][:])
        B_ = {nm: bpk[:, i:i + 1] for i, nm in enumerate(
            ("ae_b1", "ae_b2", "ad_b1", "se_b1", "se_b2", "sd_b1"))}
        b_ad2 = wp.tile([128, XD], F32)
        nc.sync.dma_start(out=b_ad2[:], in_=t["ad_b2b"][:])
        cinv_r = wp.tile([1, NL], F32)
        nc.sync.dma_start(out=cinv_r[:], in_=t["cinv"][:])
        mask_r = wp.tile([1, NL], F32)
        nc.sync.dma_start(out=mask_r[:], in_=t["mask"][:])
        ident = wp.tile([128, 128], F32)
        make_identity(nc, ident[:])
        ones = wp.tile([128, 1], F32)
        nc.gpsimd.memset(ones[:], 1.0)
        neg2 = wp.tile([128, 1], F32)
        nc.gpsimd.memset(neg2[:], -2.0)

        hT_bf = {"a": wp.tile([128, NL], BF16, name="haT_bf"),
                 "s": wp.tile([128, NL], BF16, name="hsT_bf")}
        ha_asm = {"a": wp.tile([128, NT * H], F32, name="ha_asm"),
                  "s": wp.tile([128, NT * H], F32, name="hs_asm")}
        x_asm = wp.tile([128, NT * XD], BF16)
        tblA_asm = wp.tile([128, NT * TDA], BF16)
        tblS_asm = wp.tile([128, NT * TDS], BF16)

        tblA_in = dram.tile([128, NT * TDA], BF16)
        tblA_out = dram.tile([NCORES * 128, NT * TDA], BF16, addr_space="Shared")
        tblS_in = dram.tile([128, NT * TDS], BF16)
        tblS_out = dram.tile([NCORES * 128, NT * TDS], BF16, addr_space="Shared")

        ioH = ctx1.enter_context(tc.tile_pool(name="ioH", bufs=1))
        hT_f32 = {"a": ioH.tile([128, NL], F32, name="haT_f32"),
                  "s": ioH.tile([128, NL], F32, name="hsT_f32")}
        ioS = ctx1.enter_context(tc.tile_pool(name="ioS", bufs=3))
        psA = ctx1.enter_context(tc.tile_pool(name="psA", bufs=2, space="PSUM"))
        psTr = ctx1.enter_context(tc.tile_pool(name="psTr", bufs=2, space="PSUM"))

        # ---------------- x-encoder ----------------
        with nc.named_scope("xenc"):
            xT_t = io.tile([128, 2 * NL], BF16, name="xT_t", bufs=1)
            nc.sync.dma_start(out=xT_t[:], in_=t["xT"][:])
            h1aT = psA.tile([128, NL], F32, tag="psA")
            for k in range(2):
                for j in range(2):
                    nc.tensor.matmul(h1aT[:, j * 512:(j + 1) * 512],
                                     w_ae1[:, k * H:(k + 1) * H],
                                     xT_t[:, k * NL + j * 512: k * NL + (j + 1) * 512],
                                     start=(k == 0), stop=(k == 1))
            h1aT_bf = io.tile([128, NL], BF16, name="h1aT_bf", bufs=1)
            nc.scalar.activation(h1aT_bf[:], h1aT[:], Relu, bias=B_["ae_b1"])
            haT_ps = psA.tile([128, NL], F32, tag="psA")
            for j in range(2):
                nc.tensor.matmul(haT_ps[:, j * 512:(j + 1) * 512], W["ae_W2"],
                                 h1aT_bf[:, j * 512:(j + 1) * 512],
                                 start=True, stop=True)
            nc.scalar.activation(hT_f32["a"][:], haT_ps[:], Ident, bias=B_["ae_b2"])
            nc.vector.tensor_copy(hT_bf["a"][:], hT_f32["a"][:])

            # a-side transposes; h_a output; early allgather of the h_a table
            for tt in range(NT):
                tr = psTr.tile([128, 128], F32, tag="tr")
                nc.tensor.transpose(tr[:], hT_f32["a"][:, tt * 128:(tt + 1) * 128],
                                    ident[:])
                nc.vector.tensor_copy(ha_asm["a"][:, tt * H:(tt + 1) * H], tr[:])
                nc.vector.tensor_copy(
                    tblA_asm[:, tt * TDA:tt * TDA + H], tr[:])
                sq = io.tile([128, 128], F32, name="sq", tag="sq", bufs=2)
                hsq_c = io.tile([128, 1], F32, name="hsq_c", tag="hsq_c", bufs=2)
                nc.scalar.activation(sq[:], ha_asm["a"][:, tt * H:(tt + 1) * H],
                                     Square, accum_out=hsq_c[:])
                nc.vector.tensor_copy(
                    tblS_asm[:, tt * TDS + H:tt * TDS + H + 1], hsq_c[:])
            nc.sync.dma_start(out=t["o_ha"][:], in_=ha_asm["a"][:])
            nc.sync.dma_start(out=tblA_in[:], in_=tblA_asm[:])
            nc.gpsimd.collective_compute(
                "AllGather", mybir.AluOpType.bypass,
                replica_groups=[list(range(NCORES))],
                ins=[tblA_in.opt()], outs=[tblA_out.opt()],
            )

        # ---------------- x-decoder (fills PE gaps during s-encoder) --------
        with nc.named_scope("xdec"):
            g1aT_bf = io.tile([128, NL], BF16, name="g1aT_bf", bufs=1)
            for j in range(2):
                g1 = psTr.tile([128, 512], F32, tag="tr")
                nc.tensor.matmul(g1[:], W["ad_W1"],
                                 hT_bf["a"][:, j * 512:(j + 1) * 512],
                                 start=True, stop=True)
                nc.scalar.activation(g1aT_bf[:, j * 512:(j + 1) * 512], g1[:], Relu,
                                     bias=B_["ad_b1"])
            for tt in range(NT):
                xo = psTr.tile([128, XD], F32, tag="tr")
                nc.tensor.matmul(xo[:], g1aT_bf[:, tt * 128:(tt + 1) * 128],
                                 W["ad_W2"], start=True, stop=True)
                nc.vector.tensor_tensor(x_asm[:, tt * XD:(tt + 1) * XD], xo[:],
                                        b_ad2[:], add)
            nc.sync.dma_start(out=t["o_x"][:], in_=x_asm[:])

        # ---------------- s-encoder ----------------
        with nc.named_scope("senc"):
            h1sT = psA.tile([128, NL], F32, tag="psA")
            for kg in range(SC // 4):          # 16 loads of 4 k-chunks each
                sT_t = ioS.tile([128, 4 * NL], BF16, name="sT_t", tag="sT_t")
                nc.sync.dma_start(out=sT_t[:],
                                  in_=t["sT"][:, kg * 4 * NL:(kg + 1) * 4 * NL])
                for kk in range(4):
                    k = kg * 4 + kk
                    for j in range(2):
                        nc.tensor.matmul(h1sT[:, j * 512:(j + 1) * 512],
                                         w_se1[:, k * H:(k + 1) * H],
                                         sT_t[:, kk * NL + j * 512:
                                              kk * NL + (j + 1) * 512],
                                         start=(k == 0), stop=(k == SC - 1))
            h1sT_bf = io.tile([128, NL], BF16, name="h1sT_bf", bufs=1)
            nc.scalar.activation(h1sT_bf[:], h1sT[:], Relu, bias=B_["se_b1"])
            hsT_ps = psA.tile([128, NL], F32, tag="psA")
            for j in range(2):
                nc.tensor.matmul(hsT_ps[:, j * 512:(j + 1) * 512], W["se_W2"],
                                 h1sT_bf[:, j * 512:(j + 1) * 512],
                                 start=True, stop=True)
            nc.scalar.activation(hT_f32["s"][:], hsT_ps[:], Ident, bias=B_["se_b2"])
            nc.vector.tensor_copy(hT_bf["s"][:], hT_f32["s"][:])

        # decoder-phase weights (behind the s-encoder stream in queue order)
        w_sd2 = wp.tile([128, SD], BF16)
        nc.sync.dma_start(out=w_sd2[:], in_=t["sd_W2"][:])
        b_sd2 = wp.tile([128, SD], BF16)
        nc.sync.dma_start(out=b_sd2[:], in_=t["sd_b2b"][:])

        # ------- s-side transposes; h_s output; late allgather ------------
        with nc.named_scope("tbl"):
            for tt in range(NT):
                tr = psTr.tile([128, 128], F32, tag="tr")
                nc.tensor.transpose(tr[:], hT_f32["s"][:, tt * 128:(tt + 1) * 128],
                                    ident[:])
                nc.vector.tensor_copy(ha_asm["s"][:, tt * H:(tt + 1) * H], tr[:])
                nc.vector.tensor_copy(
                    tblS_asm[:, tt * TDS:tt * TDS + H], tr[:])
                sq = io.tile([128, 128], F32, name="sq", tag="sq", bufs=2)
                hsq_c = io.tile([128, 1], F32, name="hsq_c", tag="hsq_c", bufs=2)
                nc.scalar.activation(sq[:], ha_asm["s"][:, tt * H:(tt + 1) * H],
                                     Square, accum_out=hsq_c[:])
                nc.vector.tensor_copy(
                    tblS_asm[:, tt * TDS + H + 1:tt * TDS + H + 2], hsq_c[:])
            nc.sync.dma_start(out=t["o_hs"][:], in_=ha_asm["s"][:])
            nc.sync.dma_start(out=tblS_in[:], in_=tblS_asm[:])
            nc.gpsimd.collective_compute(
                "AllGather", mybir.AluOpType.bypass,
                replica_groups=[list(range(NCORES))],
                ins=[tblS_in.opt()], outs=[tblS_out.opt()],
            )
        ctx1.close()  # free phase-1 SBUF/PSUM pools

        psD = ctx.enter_context(tc.tile_pool(name="psD", bufs=1, space="PSUM"))
        psB = ctx.enter_context(tc.tile_pool(name="psB", bufs=1, space="PSUM"))
        ioB = ctx.enter_context(tc.tile_pool(name="ioB", bufs=3))

        # ---------------- s-decoder (covers the late allgather) ------------
        with nc.named_scope("sdec"):
            g1sT_bf = io.tile([128, NL], BF16, name="g1sT_bf", bufs=1)
            for j in range(2):
                g1 = psD.tile([128, 512], F32, tag="dec", bufs=3)
                nc.tensor.matmul(g1[:], W["sd_W1"],
                                 hT_bf["s"][:, j * 512:(j + 1) * 512],
                                 start=True, stop=True)
                nc.scalar.activation(g1sT_bf[:, j * 512:(j + 1) * 512], g1[:], Relu,
                                     bias=B_["sd_b1"])
            for tt in range(NT):
                for half in range(2):
                    s_asm = ioB.tile([128, SD // 2], BF16, name="s_asm", tag="s_asm",
                                     bufs=3)
                    for jj in range(8):
                        j = half * 8 + jj
                        so = psD.tile([128, 512], F32, tag="dec", bufs=3)
                        nc.tensor.matmul(so[:], g1sT_bf[:, tt * 128:(tt + 1) * 128],
                                         w_sd2[:, j * 512:(j + 1) * 512],
                                         start=True, stop=True)
                        nc.vector.tensor_tensor(s_asm[:, jj * 512:(jj + 1) * 512],
                                                so[:], b_sd2[:, j * 512:(j + 1) * 512],
                                                add)
                    nc.sync.dma_start(
                        out=t["o_s"][:, tt * SD + half * (SD // 2):
                                     tt * SD + (half + 1) * (SD // 2)],
                        in_=s_asm[:])

        # ------- B-matmul: [m_a | m_s | q] = table^T-contracted with BT -------
        # all loads on the scalar-engine HWDGE queue (sync queue keeps stores)
        with nc.named_scope("bmm"):
            tblp = ctx.enter_context(tc.tile_pool(name="tblp", bufs=1))
            tblA_r, tblS_r = [], []
            for r in range(NCORES):
                ta = tblp.tile([128, NT * TDA], BF16, name=f"tblA_{r}")
                nc.scalar.dma_start(out=ta[:],
                                    in_=tblA_out[r * 128:(r + 1) * 128, :])
                tblA_r.append(ta)
            for r in range(NCORES):
                ts_ = tblp.tile([128, NT * TDS], BF16, name=f"tblS_{r}")
                nc.scalar.dma_start(out=ts_[:],
                                    in_=tblS_out[r * 128:(r + 1) * 128, :])
                tblS_r.append(ts_)
            dna_sb = {}
            for ab in ("a", "s"):
                dna_sb[ab] = io.tile([1, NL], F32, name=f"dna_{ab}",
                                     tag=f"dna_{ab}", bufs=1)

            for j in range(2):
                sl = slice(j * 512, (j + 1) * 512)
                m_a = psB.tile([128, 512], F32, tag="m_a")
                m_s = psB.tile([128, 512], F32, tag="m_s")
                m_q = psB.tile([2, 512], F32, tag="m_q")
                for kg in range(KC // 4):     # 16 loads of 4 k-chunks per pass
                    bt_t = ioB.tile([128, 4 * 512], BF16, name="bt_t", tag="bt_t",
                                    bufs=4)
                    base = j * KC * 512 + kg * 4 * 512
                    nc.scalar.dma_start(out=bt_t[:],
                                        in_=t["BT"][:, base:base + 4 * 512])
                    for kk in range(4):
                        k = kg * 4 + kk
                        r, tt = k // NT, k % NT
                        rhs = bt_t[:, kk * 512:(kk + 1) * 512]
                        st = dict(start=(k == 0), stop=(k == KC - 1))
                        nc.tensor.matmul(
                            m_a[:], tblA_r[r][:, tt * TDA:tt * TDA + H], rhs, **st)
                        nc.tensor.matmul(
                            m_s[:], tblS_r[r][:, tt * TDS:tt * TDS + H], rhs, **st)
                        nc.tensor.matmul(
                            m_q[:], tblS_r[r][:, tt * TDS + H:tt * TDS + H + 2],
                            rhs, **st)
                q_sb = io.tile([2, 512], F32, name="q_sb", tag="q_sb", bufs=2)
                nc.vector.tensor_copy(q_sb[:], m_q[:])

                # ---- dna / dns for this 512-node chunk ----
                # dna = mask * (hsq + cinv*(q - 2*dot)); q - 2*dot built on PE
                for ab, (qrow, m_ps) in (("a", (0, m_a)), ("s", (1, m_s))):
                    prod = io.tile([128, 512], F32, name="prod", tag="prod", bufs=2)
                    nc.vector.tensor_tensor(prod[:], m_ps[:], hT_bf[ab][:, sl], mult)
                    dotq = psD.tile([1, 512], F32, tag="dot", bufs=2)
                    nc.tensor.matmul(dotq[:], ident[0:2, qrow:qrow + 1], q_sb[:],
                                     start=True, stop=False)
                    nc.tensor.matmul(dotq[:], neg2[:], prod[:],
                                     start=False, stop=True)
                    sqT = io.tile([128, 512], F32, name="sqT", tag="sqT", bufs=2)
                    nc.scalar.activation(sqT[:], hT_bf[ab][:, sl], Square)
                    hsq_r = psD.tile([1, 512], F32, tag="dot", bufs=2)
                    nc.tensor.matmul(hsq_r[:], ones[:], sqT[:],
                                     start=True, stop=True)
                    tmp = io.tile([1, 512], F32, name="tmp", tag="tmp", bufs=2)
                    nc.vector.tensor_tensor(tmp[:], dotq[:], cinv_r[:, sl], mult)
                    nc.vector.tensor_tensor(tmp[:], tmp[:], hsq_r[:], add)
                    nc.vector.tensor_tensor(dna_sb[ab][:, sl], tmp[:],
                                            mask_r[:, sl], mult)
            nc.sync.dma_start(out=t["o_dna"][:], in_=dna_sb["a"][:])
            nc.sync.dma_start(out=t["o_dns"][:], in_=dna_sb["s"][:])


def _prep_inputs(x, s, edge_index,
                 ae_W1, ae_b1, ae_W2, ae_b2,
                 ad_W1, ad_b1, ad_W2, ad_b2,
                 se_W1, se_b1, se_W2, se_b2,
                 sd_W1, sd_b1, sd_W2, sd_b2):
    bf = ml_dtypes.bfloat16
    src = np.asarray(edge_index[0], dtype=np.int64)
    dst = np.asarray(edge_index[1], dtype=np.int64)
    cnt = np.bincount(dst, minlength=N).astype(np.float32)
    cinv = (1.0 / np.maximum(cnt, 1.0)).astype(np.float32)
    mask = (cnt > 0).astype(np.float32)
    # dense count matrix B^T[src, dst]
    BT_full = np.bincount(src * N + dst, minlength=N * N).reshape(N, N)
    BT_full = BT_full.astype(bf)

    wpack = np.concatenate([ae_W2, ad_W1, ad_W2, se_W2, sd_W1],
                           axis=1).astype(bf)            # [128, 768]
    bpack = np.stack([ae_b1, ae_b2, ad_b1, se_b1, se_b2, sd_b1],
                     axis=1).astype(np.float32)          # [128, 6]
    shared = dict(
        se_W1=_pk(se_W1.astype(bf)),
        ae_W1=_pk(ae_W1.astype(bf)),
        sd_W2=sd_W2.astype(bf),
        wpack=wpack, bpack=bpack,
        ad_b2b=np.ascontiguousarray(np.broadcast_to(
            ad_b2.astype(np.float32), (128, XD))),
        sd_b2b=np.ascontiguousarray(np.broadcast_to(
            sd_b2.astype(bf), (128, SD))),
    )
    x = np.asarray(x, dtype=np.float32)
    s = np.asarray(s, dtype=np.float32)
    in_maps = []
    for c in range(NCORES):
        rows = slice(c * NL, (c + 1) * NL)
        m = dict(shared)
        m["xT"] = _pk(np.ascontiguousarray(x[rows].T).astype(bf))
        m["sT"] = _pk(np.ascontiguousarray(s[rows].T).astype(bf))
        # BT packed as [128, (pass j, k-chunk, 512)]
        btc = np.ascontiguousarray(BT_full[:, rows])     # [8192, 1024]
        btc = btc.reshape(KC, 128, 2, 512).transpose(1, 2, 0, 3)
        m["BT"] = np.ascontiguousarray(btc.reshape(128, 2 * KC * 512))
        m["cinv"] = cinv[rows].reshape(1, NL)
        m["mask"] = mask[rows].reshape(1, NL)
        in_maps.append(m)
    return in_maps


def _run(in_maps, trace=False):
    if "nc" not in _CACHE:
        _CACHE["nc"] = _build_program()
    nc = _CACHE["nc"]
    return run_bass_kernel_spmd(nc, in_maps, core_ids=list(range(NCORES)),
                                trace=trace)


def _assemble(results):
    x_ = np.concatenate([_upk(results[c]["o_x"].astype(np.float32), XD)
                         for c in range(NCORES)], axis=0)
    s_ = np.concatenate([_upk(results[c]["o_s"].astype(np.float32), SD)
                         for c in range(NCORES)], axis=0)
    h_a = np.concatenate([_upk(results[c]["o_ha"], H)
                          for c in range(NCORES)], axis=0)
    h_s = np.concatenate([_upk(results[c]["o_hs"], H)
                          for c in range(NCORES)], axis=0)
    dna = np.concatenate([results[c]["o_dna"].reshape(NL)
                          for c in range(NCORES)], axis=0)
    dns = np.concatenate([results[c]["o_dns"].reshape(NL)
                          for c in range(NCORES)], axis=0)
    return (np.ascontiguousarray(x_, dtype=np.float32), s_,
            np.ascontiguousarray(h_a, dtype=np.float32),
            np.ascontiguousarray(h_s, dtype=np.float32),
            dna.astype(np.float32), dns.astype(np.float32))


def kernel(**inputs):
    in_maps = _prep_inputs(**inputs)
    res = _run(in_maps, trace=False)
    return _assemble(res.results)


def kernel_traced(**inputs):
    """Like kernel(), but also returns neuron-profile exec time in ns."""
    in_maps = _prep_inputs(**inputs)
    res = _run(in_maps, trace=True)
    return _assemble(res.results), res.exec_time_ns


# revision 11
# speedup vs baseline: 1.6644x; 1.0130x over previous
"""Trainium2 Bass kernel: GNN message passing (2x encoder/decoder MLP + NeighDiff).

Sharding: nodes row-sharded across 8 NeuronCores (1024 nodes/core).
The segment-mean NeighDiff is computed via the algebraic expansion
    dna_i = mask_i * (||h_i||^2 + (q_i - 2 h_i . m_i) / c_i)
with [m | q] = B @ [h | hsq], where B is the dense [dst, src] edge-count
matrix (built host-side from edge_index as part of sharding prep) and the
full node embedding table is AllGathered on-device in bf16, split into an
early gather (h_a, hidden under the s-encoder) and a late gather
(h_s | hsq_a | hsq_s, hidden under the s-decoder).

All large tensors are staged host-side in partition-major packed layouts so
every DMA moves large contiguous descriptors. Input loads for the B-matmul
ride the scalar-engine HWDGE queue so the collective-gated loads never
head-of-line-block the store stream on the sync queue.
"""

import numpy as np
import ml_dtypes

import concourse.bass as bass
import concourse.mybir as mybir
import concourse.tile as tile
from concourse import bacc
from concourse.bass_utils import run_bass_kernel_spmd
from concourse.masks import make_identity

BF16 = mybir.dt.bfloat16
F32 = mybir.dt.float32
FP8 = mybir.dt.float8e4

NCORES = 8
N = 8192          # nodes
NL = N // NCORES  # nodes per core (1024)
XD = 256          # x feature dim
H = 128           # hidden dim
SD = 8192         # s feature dim
TDA = H           # early table cols: h_a
TDS = H + 2       # late table cols: h_s | hsq_a | hsq_s
KC = N // 128     # k-chunks over source nodes (64)
SC = SD // 128    # k-chunks over s columns (64)
NT = NL // 128    # node tiles per core (8)

_CACHE = {}


def _pk(a):
    """[G*128, C] -> [128, G*C] partition-major packing."""
    g = a.shape[0] // 128
    return np.ascontiguousarray(
        a.reshape(g, 128, a.shape[1]).transpose(1, 0, 2).reshape(128, -1))


def _upk(a, c):
    """[128, G*C] -> [G*128, C] unpack."""
    g = a.shape[1] // c
    return np.ascontiguousarray(
        a.reshape(128, g, c).transpose(1, 0, 2).reshape(g * 128, c))


def _build_program():
    nc = bacc.Bacc("TRN2", target_bir_lowering=False, debug=False,
                   num_devices=NCORES)

    def din(name, shape, dt):
        return nc.dram_tensor(name, shape, dt, kind="ExternalInput").ap()

    def dout(name, shape, dt):
        return nc.dram_tensor(name, shape, dt, kind="ExternalOutput").ap()

    # per-core sharded inputs (packed layouts)
    xT = din("xT", [128, 2 * NL], BF16)       # x rows transposed, packed
    sT = din("sT", [128, SC * NL], BF16)      # s rows transposed, packed by k-chunk
    BT = din("BT", [128, 2 * KC * 512], FP8)  # count matrix, packed [j, k, 512]
    cinv = din("cinv", [1, NL], F32)
    mask = din("mask", [1, NL], F32)
    se_W1 = din("se_W1", [128, SC * H], BF16)  # packed k-chunks
    ae_W1 = din("ae_W1", [128, 2 * H], BF16)
    sd_W2 = din("sd_W2", [128, SD], BF16)
    # small weights packed: ae_W2 | ad_W1 | ad_W2 | se_W2 | sd_W1
    wpack = din("wpack", [128, 768], BF16)
    bpack = din("bpack", [128, 6], F32)       # ae_b1|ae_b2|ad_b1|se_b1|se_b2|sd_b1
    ad_b2b = din("ad_b2b", [128, XD], F32)    # broadcast decoder output biases
    sd_b2b = din("sd_b2b", [128, SD], BF16)

    # per-core outputs (packed layouts, unpacked on host)
    o_x = dout("o_x", [128, NT * XD], BF16)
    o_s = dout("o_s", [128, NT * SD], BF16)
    o_ha = dout("o_ha", [128, NT * H], F32)
    o_hs = dout("o_hs", [128, NT * H], F32)
    o_dna = dout("o_dna", [1, NL], F32)
    o_dns = dout("o_dns", [1, NL], F32)

    with tile.TileContext(nc) as tc:
        _emit(nc, tc, locals())
    nc.compile()
    return nc


def _emit(nc, tc, t):
    Relu = mybir.ActivationFunctionType.Relu
    Ident = mybir.ActivationFunctionType.Identity
    Square = mybir.ActivationFunctionType.Square
    add = mybir.AluOpType.add
    mult = mybir.AluOpType.mult

    import contextlib
    ctx = contextlib.ExitStack()
    ctx1 = contextlib.ExitStack()  # phase-1 pools, closed before phase 2
    with ctx:
        wp = ctx.enter_context(tc.tile_pool(name="wp", bufs=1))
        io = ctx.enter_context(tc.tile_pool(name="io", bufs=2))
        dram = ctx.enter_context(tc.tile_pool(name="dram", bufs=1, space="DRAM"))

        # --- phase-1 critical weights ---
        w_se1 = wp.tile([128, SC * H], BF16)
        nc.sync.dma_start(out=w_se1[:], in_=t["se_W1"][:])
        w_ae1 = wp.tile([128, 2 * H], BF16)
        nc.sync.dma_start(out=w_ae1[:], in_=t["ae_W1"][:])
        wpk = wp.tile([128, 768], BF16)
        nc.sync.dma_start(out=wpk[:], in_=t["wpack"][:])
        W = {"ae_W2": wpk[:, 0:128], "ad_W1": wpk[:, 128:256],
             "ad_W2": wpk[:, 256:512], "se_W2": wpk[:, 512:640],
             "sd_W1": wpk[:, 640:768]}
        bpk = wp.tile([128, 6], F32)
        nc.sync.dma_start(out=bpk[:], in_=t["bpack"][:])
        B_ = {nm: bpk[:, i:i + 1] for i, nm in enumerate(
            ("ae_b1", "ae_b2", "ad_b1", "se_b1", "se_b2", "sd_b1"))}
        b_ad2 = wp.tile([128, XD], F32)
        nc.sync.dma_start(out=b_ad2[:], in_=t["ad_b2b"][:])
        cinv_r = wp.tile([1, NL], F32)
        nc.sync.dma_start(out=cinv_r[:], in_=t["cinv"][:])
        mask_r = wp.tile([1, NL], F32)
        nc.sync.dma_start(out=mask_r[:], in_=t["mask"][:])
        ident = wp.tile([128, 128], F32)
        make_identity(nc, ident[:])
        ones = wp.tile([128, 1], F32)
        nc.gpsimd.memset(ones[:], 1.0)
        neg2 = wp.tile([128, 1], F32)
        nc.gpsimd.memset(neg2[:], -2.0)
        ones1r = wp.tile([1, 128], BF16)
        nc.gpsimd.memset(ones1r[:], 1.0)

        hT_bf = {"a": wp.tile([128, NL], BF16, name="haT_bf"),
                 "s": wp.tile([128, NL], BF16, name="hsT_bf")}
        ha_asm = {"a": wp.tile([128, NT * H], F32, name="ha_asm"),
                  "s": wp.tile([128, NT * H], F32, name="hs_asm")}
        x_asm = wp.tile([128, NT * XD], BF16)
        tblA_asm = wp.tile([128, NT * TDA], BF16)
        tblS_asm = wp.tile([128, NT * TDS], BF16)

        tblA_in = dram.tile([128, NT * TDA], BF16)
        tblA_out = dram.tile([NCORES * 128, NT * TDA], BF16, addr_space="Shared")
        tblS_in = dram.tile([128, NT * TDS], BF16)
        tblS_out = dram.tile([NCORES * 128, NT * TDS], BF16, addr_space="Shared")

        ioH = ctx1.enter_context(tc.tile_pool(name="ioH", bufs=1))
        hT_f32 = {"a": ioH.tile([128, NL], F32, name="haT_f32"),
                  "s": ioH.tile([128, NL], F32, name="hsT_f32")}
        ioS = ctx1.enter_context(tc.tile_pool(name="ioS", bufs=3))
        psA = ctx1.enter_context(tc.tile_pool(name="psA", bufs=2, space="PSUM"))
        psTr = ctx1.enter_context(tc.tile_pool(name="psTr", bufs=2, space="PSUM"))

        # ---------------- x-encoder ----------------
        with nc.named_scope("xenc"):
            xT_t = io.tile([128, 2 * NL], BF16, name="xT_t", bufs=1)
            nc.sync.dma_start(out=xT_t[:], in_=t["xT"][:])
            h1aT = psA.tile([128, NL], F32, tag="psA")
            for k in range(2):
                for j in range(2):
                    nc.tensor.matmul(h1aT[:, j * 512:(j + 1) * 512],
                                     w_ae1[:, k * H:(k + 1) * H],
                                     xT_t[:, k * NL + j * 512: k * NL + (j + 1) * 512],
                                     start=(k == 0), stop=(k == 1))
            h1aT_bf = io.tile([128, NL], BF16, name="h1aT_bf", bufs=1)
            nc.scalar.activation(h1aT_bf[:], h1aT[:], Relu, bias=B_["ae_b1"])
            haT_ps = psA.tile([128, NL], F32, tag="psA")
            for j in range(2):
                nc.tensor.matmul(haT_ps[:, j * 512:(j + 1) * 512], W["ae_W2"],
                                 h1aT_bf[:, j * 512:(j + 1) * 512],
                                 start=True, stop=True)
            nc.scalar.activation(hT_f32["a"][:], haT_ps[:], Ident, bias=B_["ae_b2"])
            nc.vector.tensor_copy(hT_bf["a"][:], hT_f32["a"][:])

            # a-side transposes; h_a output; early allgather of the h_a table
            for tt in range(NT):
                tr = psTr.tile([128, 128], F32, tag="tr")
                nc.tensor.transpose(tr[:], hT_f32["a"][:, tt * 128:(tt + 1) * 128],
                                    ident[:])
                nc.vector.tensor_copy(ha_asm["a"][:, tt * H:(tt + 1) * H], tr[:])
                nc.vector.tensor_copy(
                    tblA_asm[:, tt * TDA:tt * TDA + H], tr[:])
                sq = io.tile([128, 128], F32, name="sq", tag="sq", bufs=2)
                hsq_c = io.tile([128, 1], F32, name="hsq_c", tag="hsq_c", bufs=2)
                nc.scalar.activation(sq[:], ha_asm["a"][:, tt * H:(tt + 1) * H],
                                     Square, accum_out=hsq_c[:])
                nc.vector.tensor_copy(
                    tblS_asm[:, tt * TDS + H:tt * TDS + H + 1], hsq_c[:])
            nc.sync.dma_start(out=t["o_ha"][:], in_=ha_asm["a"][:])
            nc.sync.dma_start(out=tblA_in[:], in_=tblA_asm[:])
            nc.gpsimd.collective_compute(
                "AllGather", mybir.AluOpType.bypass,
                replica_groups=[list(range(NCORES))],
                ins=[tblA_in.opt()], outs=[tblA_out.opt()],
            )

        # ---------------- x-decoder (fills PE gaps during s-encoder) --------
        with nc.named_scope("xdec"):
            g1aT_bf = io.tile([128, NL], BF16, name="g1aT_bf", bufs=1)
            for j in range(2):
                g1 = psTr.tile([128, 512], F32, tag="tr")
                nc.tensor.matmul(g1[:], W["ad_W1"],
                                 hT_bf["a"][:, j * 512:(j + 1) * 512],
                                 start=True, stop=True)
                nc.scalar.activation(g1aT_bf[:, j * 512:(j + 1) * 512], g1[:], Relu,
                                     bias=B_["ad_b1"])
            for tt in range(NT):
                xo = psTr.tile([128, XD], F32, tag="tr")
                nc.tensor.matmul(xo[:], g1aT_bf[:, tt * 128:(tt + 1) * 128],
                                 W["ad_W2"], start=True, stop=True)
                nc.vector.tensor_tensor(x_asm[:, tt * XD:(tt + 1) * XD], xo[:],
                                        b_ad2[:], add)
            nc.sync.dma_start(out=t["o_x"][:], in_=x_asm[:])

        # ---------------- s-encoder ----------------
        with nc.named_scope("senc"):
            h1sT = psA.tile([128, NL], F32, tag="psA")
            for kg in range(SC // 4):          # 16 loads of 4 k-chunks each
                sT_t = ioS.tile([128, 4 * NL], BF16, name="sT_t", tag="sT_t")
                nc.sync.dma_start(out=sT_t[:],
                                  in_=t["sT"][:, kg * 4 * NL:(kg + 1) * 4 * NL])
                for kk in range(4):
                    k = kg * 4 + kk
                    for j in range(2):
                        nc.tensor.matmul(h1sT[:, j * 512:(j + 1) * 512],
                                         w_se1[:, k * H:(k + 1) * H],
                                         sT_t[:, kk * NL + j * 512:
                                              kk * NL + (j + 1) * 512],
                                         start=(k == 0), stop=(k == SC - 1))
            h1sT_bf = io.tile([128, NL], BF16, name="h1sT_bf", bufs=1)
            nc.scalar.activation(h1sT_bf[:], h1sT[:], Relu, bias=B_["se_b1"])
            hsT_ps = psA.tile([128, NL], F32, tag="psA")
            for j in range(2):
                nc.tensor.matmul(hsT_ps[:, j * 512:(j + 1) * 512], W["se_W2"],
                                 h1sT_bf[:, j * 512:(j + 1) * 512],
                                 start=True, stop=True)
            nc.scalar.activation(hT_f32["s"][:], hsT_ps[:], Ident, bias=B_["se_b2"])
            nc.vector.tensor_copy(hT_bf["s"][:], hT_f32["s"][:])

        # decoder-phase weights (behind the s-encoder stream in queue order)
        w_sd2 = wp.tile([128, SD], BF16)
        nc.sync.dma_start(out=w_sd2[:], in_=t["sd_W2"][:])
        b_sd2 = wp.tile([128, SD], BF16)
        nc.sync.dma_start(out=b_sd2[:], in_=t["sd_b2b"][:])

        # ------- s-side transposes; h_s output; late allgather ------------
        with nc.named_scope("tbl"):
            for tt in range(NT):
                tr = psTr.tile([128, 128], F32, tag="tr")
                nc.tensor.transpose(tr[:], hT_f32["s"][:, tt * 128:(tt + 1) * 128],
                                    ident[:])
                nc.vector.tensor_copy(ha_asm["s"][:, tt * H:(tt + 1) * H], tr[:])
                nc.vector.tensor_copy(
                    tblS_asm[:, tt * TDS:tt * TDS + H], tr[:])
                sq = io.tile([128, 128], F32, name="sq", tag="sq", bufs=2)
                hsq_c = io.tile([128, 1], F32, name="hsq_c", tag="hsq_c", bufs=2)
                nc.scalar.activation(sq[:], ha_asm["s"][:, tt * H:(tt + 1) * H],
                                     Square, accum_out=hsq_c[:])
                nc.vector.tensor_copy(
                    tblS_asm[:, tt * TDS + H + 1:tt * TDS + H + 2], hsq_c[:])
            nc.sync.dma_start(out=t["o_hs"][:], in_=ha_asm["s"][:])
            nc.sync.dma_start(out=tblS_in[:], in_=tblS_asm[:])
            nc.gpsimd.collective_compute(
                "AllGather", mybir.AluOpType.bypass,
                replica_groups=[list(range(NCORES))],
                ins=[tblS_in.opt()], outs=[tblS_out.opt()],
            )
        ctx1.close()  # free phase-1 SBUF/PSUM pools

        psD = ctx.enter_context(tc.tile_pool(name="psD", bufs=1, space="PSUM"))
        psB = ctx.enter_context(tc.tile_pool(name="psB", bufs=1, space="PSUM"))
        ioB = ctx.enter_context(tc.tile_pool(name="ioB", bufs=3))

        # ---------------- s-decoder (covers the late allgather) ------------
        with nc.named_scope("sdec"):
            g1sT_bf = io.tile([128, NL], BF16, name="g1sT_bf", bufs=1)
            for j in range(2):
                g1 = psD.tile([128, 512], F32, tag="dec", bufs=3)
                nc.tensor.matmul(g1[:], W["sd_W1"],
                                 hT_bf["s"][:, j * 512:(j + 1) * 512],
                                 start=True, stop=True)
                nc.scalar.activation(g1sT_bf[:, j * 512:(j + 1) * 512], g1[:], Relu,
                                     bias=B_["sd_b1"])
            for tt in range(NT):
                for half in range(2):
                    s_asm = ioB.tile([128, SD // 2], BF16, name="s_asm", tag="s_asm",
                                     bufs=2)
                    for jj in range(8):
                        j = half * 8 + jj
                        so = psD.tile([128, 512], F32, tag="dec", bufs=3)
                        if j % 2 == 0:
                            nc.tensor.matmul(so[:],
                                             g1sT_bf[:, tt * 128:(tt + 1) * 128],
                                             w_sd2[:, j * 512:(j + 1) * 512],
                                             start=True, stop=True)
                            nc.vector.tensor_tensor(
                                s_asm[:, jj * 512:(jj + 1) * 512], so[:],
                                b_sd2[:, j * 512:(j + 1) * 512], add)
                        else:
                            # bias via PE rank-1 accumulate; pure copy on ACT
                            nc.tensor.matmul(so[:],
                                             g1sT_bf[:, tt * 128:(tt + 1) * 128],
                                             w_sd2[:, j * 512:(j + 1) * 512],
                                             start=True, stop=False)
                            nc.tensor.matmul(so[:], ones1r[:],
                                             b_sd2[0:1, j * 512:(j + 1) * 512],
                                             start=False, stop=True)
                            nc.scalar.copy(s_asm[:, jj * 512:(jj + 1) * 512], so[:])
                    nc.sync.dma_start(
                        out=t["o_s"][:, tt * SD + half * (SD // 2):
                                     tt * SD + (half + 1) * (SD // 2)],
                        in_=s_asm[:])

        # ------- B-matmul: [m_a | m_s | q] = table^T-contracted with BT -------
        # all loads on the scalar-engine HWDGE queue (sync queue keeps stores)
        with nc.named_scope("bmm"):
            tblp = ctx.enter_context(tc.tile_pool(name="tblp", bufs=1))
            tblA_r, tblS_r = [], []
            for r in range(NCORES):
                ta = tblp.tile([128, NT * TDA], BF16, name=f"tblA_{r}")
                nc.gpsimd.dma_start(out=ta[:],
                                    in_=tblA_out[r * 128:(r + 1) * 128, :])
                tblA_r.append(ta)
            for r in range(NCORES):
                ts_ = tblp.tile([128, NT * TDS], BF16, name=f"tblS_{r}")
                nc.gpsimd.dma_start(out=ts_[:],
                                    in_=tblS_out[r * 128:(r + 1) * 128, :])
                tblS_r.append(ts_)
            dna_sb = {}
            for ab in ("a", "s"):
                dna_sb[ab] = io.tile([1, NL], F32, name=f"dna_{ab}",
                                     tag=f"dna_{ab}", bufs=1)

            for j in range(2):
                sl = slice(j * 512, (j + 1) * 512)
                m_a = psB.tile([128, 512], F32, tag="m_a")
                m_s = psB.tile([128, 512], F32, tag="m_s")
                m_q = psB.tile([2, 512], F32, tag="m_q")
                for kg in range(KC // 4):     # 16 loads of 4 k-chunks per pass
                    bt_t = ioB.tile([128, 4 * 512], BF16, name="bt_t", tag="bt_t",
                                    bufs=6)
                    base = j * KC * 512 + kg * 4 * 512
                    nc.gpsimd.dma_start(out=bt_t[:],
                                        in_=t["BT"][:, base:base + 4 * 512])
                    for kk in range(4):
                        k = kg * 4 + kk
                        r, tt = k // NT, k % NT
                        rhs = bt_t[:, kk * 512:(kk + 1) * 512]
                        st = dict(start=(k == 0), stop=(k == KC - 1))
                        nc.tensor.matmul(
                            m_a[:], tblA_r[r][:, tt * TDA:tt * TDA + H], rhs, **st)
                        nc.tensor.matmul(
                            m_s[:], tblS_r[r][:, tt * TDS:tt * TDS + H], rhs, **st)
                        nc.tensor.matmul(
                            m_q[:], tblS_r[r][:, tt * TDS + H:tt * TDS + H + 2],
                            rhs, **st)
                q_sb = io.tile([2, 512], F32, name="q_sb", tag="q_sb", bufs=2)
                nc.vector.tensor_copy(q_sb[:], m_q[:])

                # ---- dna / dns for this 512-node chunk ----
                # dna = mask * (hsq + cinv*(q - 2*dot)); q - 2*dot built on PE
                for ab, (qrow, m_ps) in (("a", (0, m_a)), ("s", (1, m_s))):
                    prod = io.tile([128, 512], F32, name="prod", tag="prod", bufs=2)
                    nc.vector.tensor_tensor(prod[:], m_ps[:], hT_bf[ab][:, sl], mult)
                    dotq = psD.tile([1, 512], F32, tag="dot", bufs=2)
                    nc.tensor.matmul(dotq[:], ident[0:2, qrow:qrow + 1], q_sb[:],
                                     start=True, stop=False)
                    nc.tensor.matmul(dotq[:], neg2[:], prod[:],
                                     start=False, stop=True)
                    sqT = io.tile([128, 512], F32, name="sqT", tag="sqT", bufs=2)
                    nc.scalar.activation(sqT[:], hT_bf[ab][:, sl], Square)
                    hsq_r = psD.tile([1, 512], F32, tag="dot", bufs=2)
                    nc.tensor.matmul(hsq_r[:], ones[:], sqT[:],
                                     start=True, stop=True)
                    tmp = io.tile([1, 512], F32, name="tmp", tag="tmp", bufs=2)
                    nc.vector.tensor_tensor(tmp[:], dotq[:], cinv_r[:, sl], mult)
                    nc.vector.tensor_tensor(tmp[:], tmp[:], hsq_r[:], add)
                    nc.vector.tensor_tensor(dna_sb[ab][:, sl], tmp[:],
                                            mask_r[:, sl], mult)
            nc.sync.dma_start(out=t["o_dna"][:], in_=dna_sb["a"][:])
            nc.sync.dma_start(out=t["o_dns"][:], in_=dna_sb["s"][:])


def _prep_inputs(x, s, edge_index,
                 ae_W1, ae_b1, ae_W2, ae_b2,
                 ad_W1, ad_b1, ad_W2, ad_b2,
                 se_W1, se_b1, se_W2, se_b2,
                 sd_W1, sd_b1, sd_W2, sd_b2):
    bf = ml_dtypes.bfloat16
    src = np.asarray(edge_index[0], dtype=np.int64)
    dst = np.asarray(edge_index[1], dtype=np.int64)
    cnt = np.bincount(dst, minlength=N).astype(np.float32)
    cinv = (1.0 / np.maximum(cnt, 1.0)).astype(np.float32)
    mask = (cnt > 0).astype(np.float32)
    # dense count matrix B^T[src, dst]
    BT_full = np.bincount(src * N + dst, minlength=N * N).reshape(N, N)
    assert BT_full.max() <= 16, "edge multiplicity exceeds exact fp8 range"
    BT_full = BT_full.astype(ml_dtypes.float8_e4m3fn)

    wpack = np.concatenate([ae_W2, ad_W1, ad_W2, se_W2, sd_W1],
                           axis=1).astype(bf)            # [128, 768]
    bpack = np.stack([ae_b1, ae_b2, ad_b1, se_b1, se_b2, sd_b1],
                     axis=1).astype(np.float32)          # [128, 6]
    shared = dict(
        se_W1=_pk(se_W1.astype(bf)),
        ae_W1=_pk(ae_W1.astype(bf)),
        sd_W2=sd_W2.astype(bf),
        wpack=wpack, bpack=bpack,
        ad_b2b=np.ascontiguousarray(np.broadcast_to(
            ad_b2.astype(np.float32), (128, XD))),
        sd_b2b=np.ascontiguousarray(np.broadcast_to(
            sd_b2.astype(bf), (128, SD))),
    )
    x = np.asarray(x, dtype=np.float32)
    s = np.asarray(s, dtype=np.float32)
    in_maps = []
    for c in range(NCORES):
        rows = slice(c * NL, (c + 1) * NL)
        m = dict(shared)
        m["xT"] = _pk(np.ascontiguousarray(x[rows].T).astype(bf))
        m["sT"] = _pk(np.ascontiguousarray(s[rows].T).astype(bf))
        # BT packed as [128, (pass j, k-chunk, 512)]
        btc = np.ascontiguousarray(BT_full[:, rows])     # [8192, 1024]
        btc = btc.reshape(KC, 128, 2, 512).transpose(1, 2, 0, 3)
        m["BT"] = np.ascontiguousarray(btc.reshape(128, 2 * KC * 512))
        m["cinv"] = cinv[rows].reshape(1, NL)
        m["mask"] = mask[rows].reshape(1, NL)
        in_maps.append(m)
    return in_maps


def _run(in_maps, trace=False):
    if "nc" not in _CACHE:
        _CACHE["nc"] = _build_program()
    nc = _CACHE["nc"]
    return run_bass_kernel_spmd(nc, in_maps, core_ids=list(range(NCORES)),
                                trace=trace)


def _assemble(results):
    x_ = np.concatenate([_upk(results[c]["o_x"].astype(np.float32), XD)
                         for c in range(NCORES)], axis=0)
    s_ = np.concatenate([_upk(results[c]["o_s"].astype(np.float32), SD)
                         for c in range(NCORES)], axis=0)
    h_a = np.concatenate([_upk(results[c]["o_ha"], H)
                          for c in range(NCORES)], axis=0)
    h_s = np.concatenate([_upk(results[c]["o_hs"], H)
                          for c in range(NCORES)], axis=0)
    dna = np.concatenate([results[c]["o_dna"].reshape(NL)
                          for c in range(NCORES)], axis=0)
    dns = np.concatenate([results[c]["o_dns"].reshape(NL)
                          for c in range(NCORES)], axis=0)
    return (np.ascontiguousarray(x_, dtype=np.float32), s_,
            np.ascontiguousarray(h_a, dtype=np.float32),
            np.ascontiguousarray(h_s, dtype=np.float32),
            dna.astype(np.float32), dns.astype(np.float32))


def kernel(**inputs):
    in_maps = _prep_inputs(**inputs)
    res = _run(in_maps, trace=False)
    return _assemble(res.results)


def kernel_traced(**inputs):
    """Like kernel(), but also returns neuron-profile exec time in ns."""
    in_maps = _prep_inputs(**inputs)
    res = _run(in_maps, trace=True)
    return _assemble(res.results), res.exec_time_ns
